# revision 14
# baseline (speedup 1.0000x reference)
"""GPT forward on 8 Trainium2 NeuronCores.

Sharding: DP=2 (batch) x TP=4 (heads / FFN / vocab).
Cores 0-3 run batch 0, cores 4-7 batch 1; within each group of 4 the
attention heads (16 -> 4/core), FFN hidden (4096 -> 1024/core) and the
LM-head vocab (32000 -> 8000/core) are sharded Megatron-style with a
bf16 AllReduce after the attention out-projection and the FFN
down-projection.

On-device layout: the residual stream x lives feature-major ([d, t]) in
fp32 SBUF. All matmuls run in bf16 with fp32 PSUM accumulation.
LayerNorm statistics are computed with ones-vector matmuls on the
tensor engine (cross-partition sums); scale/bias of every LayerNorm are
folded into the following weights on the host. Attention computes the
score matrix transposed (s^T = k^T-major) so that no on-chip transposes
are needed anywhere; the softmax denominator comes for free from an
extra ones-column appended to V, and 1/l is applied per-head with a
partition-broadcast + one DVE multiply. exp() runs without max
subtraction (scores are O(1) by construction for this model family).
"""
import sys

sys.path.insert(0, "/opt/trn_rl_repo")

import os
import numpy as np
import ml_dtypes

import concourse.bass as bass
import concourse.mybir as mybir
import concourse.tile as tile
from concourse import bacc
from concourse.bass_utils import run_bass_kernel_spmd

BF = mybir.dt.bfloat16
F32 = mybir.dt.float32
bf16 = ml_dtypes.bfloat16

B, T, D, H, V = 2, 1024, 1024, 16, 32000
L = int(os.environ.get("KERNEL_LAYERS", "8"))
DH = D // H              # 64
TP, DP = 4, 2
NH = H // TP             # 4 heads per core
QH = NH * DH             # 256 q/k/v width per core
VW = NH * 65             # v_aug width (64 cols + ones col per head)
FF = 4 * D // TP         # 1024 ffn shard
VS = V // TP             # 8000 vocab shard
NCH = D // 128           # 8 d-chunks
FCH = FF // 128          # 8 ffn chunks
TC = T // 128            # 8 t-chunks
NVC = 16                 # lm-head vocab chunks
VCW = VS // NVC          # 500
EPS = 1e-5
MASKVAL = -200.0

ActF = mybir.ActivationFunctionType
Alu = mybir.AluOpType

_CACHE = {}


def _build_nc():
    nc = bacc.Bacc()

    x0 = nc.declare_dram_parameter("x0", [D, T], F32, isOutput=False)
    maskT = nc.declare_dram_parameter("maskT", [128, 128], F32, isOutput=False)
    wqk = [nc.declare_dram_parameter(f"wqk{i}", [D, 2 * QH], BF, isOutput=False) for i in range(L)]
    wv = [nc.declare_dram_parameter(f"wv{i}", [D, VW], BF, isOutput=False) for i in range(L)]
    wo = [nc.declare_dram_parameter(f"wo{i}", [QH, D], BF, isOutput=False) for i in range(L)]
    w1 = [nc.declare_dram_parameter(f"w1{i}", [D, FF], BF, isOutput=False) for i in range(L)]
    w2 = [nc.declare_dram_parameter(f"w2{i}", [FF, D], BF, isOutput=False) for i in range(L)]
    bqk_d = nc.declare_dram_parameter("bqk", [L, 2 * QH], F32, isOutput=False)
    bvr_d = nc.declare_dram_parameter("bvr", [L, VW], BF, isOutput=False)
    bo_d = nc.declare_dram_parameter("bo", [L, D], F32, isOutput=False)
    b1_d = nc.declare_dram_parameter("b1", [L, FF], F32, isOutput=False)
    b2_d = nc.declare_dram_parameter("b2", [L, D], F32, isOutput=False)
    wl_d = nc.declare_dram_parameter("wl", [D, VS], BF, isOutput=False)
    blr_d = nc.declare_dram_parameter("blr", [1, VS], BF, isOutput=False)
    out_d = nc.declare_dram_parameter("out", [T, VS], F32, isOutput=True)

    arin_a = [nc.dram_tensor(f"arin_a{i}", [D, T], BF) for i in range(L)]
    arout_a = [nc.dram_tensor(f"arout_a{i}", [D, T], BF) for i in range(L)]
    arin_f = [nc.dram_tensor(f"arin_f{i}", [D, T], BF) for i in range(L)]
    arout_f = [nc.dram_tensor(f"arout_f{i}", [D, T], BF) for i in range(L)]
    RG = [[0, 1, 2, 3], [4, 5, 6, 7]]

    from contextlib import ExitStack
    with tile.TileContext(nc) as tc, ExitStack() as _stk:
        sb = _stk.enter_context(tc.tile_pool(name="sb", bufs=1))
        ps = _stk.enter_context(tc.tile_pool(name="ps", bufs=1, space="PSUM"))

        def sbt(shape, dt, tag, bufs):
            return sb.tile(shape, dt, tag=tag, bufs=bufs, name=tag)

        def pst(shape, tag, bufs):
            return ps.tile(shape, F32, tag=tag, bufs=bufs, name=tag)

        # ---- persistent tiles / constants ----
        x_t = sbt([128, NCH, T], F32, "x", 1)            # residual, feature-major
        ones_r = sbt([1, 128], BF, "ones_r", 1)          # ones row (K=1 lhsT)
        ones_c = sbt([128, 1], BF, "ones_c", 1)          # ones col (stats lhsT)
        mask_t = sbt([128, 128], F32, "mask", 1)
        eps_t = sbt([1, 1], F32, "eps", 1)
        bqk_t = sbt([128, L, 4], F32, "bqk", 1)
        bvr_t = sbt([1, L, VW], BF, "bvr", 1)
        bo_t = sbt([128, L, NCH], F32, "bo", 1)
        b1_t = sbt([128, L, FCH], F32, "b1", 1)
        b2_t = sbt([128, L, NCH], F32, "b2", 1)

        nc.vector.memset(ones_r[:], 1.0)
        nc.vector.memset(ones_c[:], 1.0)
        nc.vector.memset(eps_t[:], EPS)
        nc.sync.dma_start(out=mask_t[:], in_=maskT[:])
        nc.sync.dma_start(out=x_t[:], in_=x0.rearrange("(c p) t -> p c t", p=128))
        nc.sync.dma_start(out=bqk_t[:], in_=bqk_d.rearrange("l (m p) -> p l m", p=128))
        nc.sync.dma_start(out=bvr_t[:], in_=bvr_d.rearrange("(o l) w -> o l w", o=1))
        nc.sync.dma_start(out=bo_t[:], in_=bo_d.rearrange("l (m p) -> p l m", p=128))
        nc.sync.dma_start(out=b1_t[:], in_=b1_d.rearrange("l (m p) -> p l m", p=128))
        nc.sync.dma_start(out=b2_t[:], in_=b2_d.rearrange("l (m p) -> p l m", p=128))

        def layernorm(h_t, li):
            """LN of x into h_t ([128, NCH, T] bf16), no scale/bias."""
            stat = pst([65, T], "stat", 1)  # p0 = sum(x), p32 = sum(x^2), p64 = scratch
            for c in range(NCH):
                xb = sbt([128, T], BF, "xb", 2)
                x2 = sbt([128, T], BF, "x2", 2)
                nc.gpsimd.tensor_copy(xb[:], x_t[:, c, :])
                nc.scalar.square(x2[:], x_t[:, c, :])
                for n in range(2):
                    nc.tensor.matmul(stat[0:1, n * 512:(n + 1) * 512], ones_c[:],
                                     xb[:, n * 512:(n + 1) * 512],
                                     start=(c == 0), stop=(c == NCH - 1),
                                     skip_group_check=True)
                    nc.tensor.matmul(stat[32:33, n * 512:(n + 1) * 512], ones_c[:],
                                     x2[:, n * 512:(n + 1) * 512],
                                     start=(c == 0), stop=(c == NCH - 1),
                                     skip_group_check=True)
            nm = sbt([1, T], F32, "ln_nm", 1)     # -mean
            msq = sbt([1, T], F32, "ln_msq", 1)   # E[x^2] -> var -> std
            rstd = sbt([1, T], F32, "ln_rstd", 1)
            sqm = stat[64:65, :]                  # psum scratch: mean^2, -mean*rstd
            nc.vector.tensor_scalar_mul(nm[:], stat[0:1, :], -1.0 / D)
            nc.vector.tensor_scalar_mul(msq[:], stat[32:33, :], 1.0 / D)
            nc.vector.tensor_mul(sqm, nm[:], nm[:])
            nc.vector.tensor_sub(msq[:], msq[:], sqm)
            nc.scalar.activation(msq[:], msq[:], ActF.Sqrt, bias=eps_t[:])
            nc.vector.reciprocal(rstd[:], msq[:])
            nc.vector.tensor_mul(sqm, nm[:], rstd[:])
            stb = sbt([1, 2, T], BF, "lnstb", 1)
            nc.vector.tensor_copy(stb[0:1, 0, :], sqm)
            nc.vector.tensor_copy(stb[0:1, 1, :], rstd[:])
            nm_b = sbt([128, T], BF, "nm_b", 2)
            rs_b = sbt([128, T], BF, "rs_b", 2)
            nc.gpsimd.partition_broadcast(nm_b[:], stb[0:1, 0, :])
            nc.gpsimd.partition_broadcast(rs_b[:], stb[0:1, 1, :])
            for c in range(NCH):
                tmp = sbt([128, T], BF, "lntmp", 2)
                nc.vector.tensor_mul(tmp[:], x_t[:, c, :], rs_b[:])
                nc.vector.tensor_add(h_t[:, c, :], tmp[:], nm_b[:])

        def allreduce_add(src_sb_fn, arin, arout, bias_t, li):
            """Evacuate 8 fm psum chunks (via src_sb_fn -> bf16 tiles), DMA to
            arin, AllReduce into arout, then x += arout."""
            for m in range(NCH):
                st = src_sb_fn(m)
                nc.sync.dma_start(out=arin[m * 128:(m + 1) * 128, :], in_=st[:])
            nc.gpsimd.collective_compute(
                "AllReduce", Alu.add, replica_groups=RG,
                ins=[arin[:]], outs=[arout[:]])
            for c in range(NCH):
                aro = sbt([128, T], BF, "arld", 2)
                nc.sync.dma_start(out=aro[:], in_=arout.rearrange("(c p) t -> p c t", p=128)[:, c, :])
                nc.vector.tensor_add(x_t[:, c, :], x_t[:, c, :], aro[:])

        for li in range(L):
            # ---------------- attention ----------------
            h_t = sbt([128, NCH, T], BF, "h", 1)
            layernorm(h_t, li)

            wqk_t = sbt([128, NCH, 2 * QH], BF, "wqk", 1)
            wv_t = sbt([128, NCH, VW], BF, "wv", 1)
            wo_t = sbt([128, 2, D], BF, "wo", 1)
            nc.sync.dma_start(out=wqk_t[:], in_=wqk[li].rearrange("(c p) n -> p c n", p=128))
            nc.sync.dma_start(out=wv_t[:], in_=wv[li].rearrange("(c p) n -> p c n", p=128))
            nc.sync.dma_start(out=wo_t[:], in_=wo[li].rearrange("(c p) n -> p c n", p=128))

            # q/k feature-major: qk_t rows x [m, t]; m=0,1 -> q (heads 2m..),
            # m=2,3 -> k
            qk_t = sbt([128, 4, T], BF, "qk", 1)
            for m in range(4):
                for n in range(2):
                    qp = pst([128, 512], "mm", 3)
                    for c in range(NCH):
                        nc.tensor.matmul(qp[:], wqk_t[:, c, m * 128:(m + 1) * 128],
                                         h_t[:, c, n * 512:(n + 1) * 512],
                                         start=(c == 0), stop=(c == NCH - 1))
                    nc.scalar.activation(qk_t[:, m, n * 512:(n + 1) * 512], qp[:],
                                         ActF.Identity, bias=bqk_t[:, li, m:m + 1])
            # v token-major (with ones col per head): [t-chunk][128, VW]
            v_t = sbt([128, TC, VW], BF, "v", 1)
            for tc_ in range(TC):
                vp = pst([128, VW], "small", 3)
                nc.tensor.matmul(vp[:], ones_r[0:1, :], bvr_t[0:1, li, :],
                                 start=True, stop=False)
                for c in range(NCH):
                    nc.tensor.matmul(vp[:], h_t[:, c, tc_ * 128:(tc_ + 1) * 128],
                                     wv_t[:, c, :],
                                     start=False, stop=(c == NCH - 1))
                nc.vector.tensor_copy(v_t[:, tc_, :], vp[:])

            # scores^T + softmax + attnV, per head
            y_t = sbt([128, 2, T], BF, "y", 1)
            for hh in range(NH):
                qrow, qm = (hh % 2) * 64, hh // 2
                km = 2 + hh // 2
                eTs = []
                for kc in range(TC):
                    w = (TC - kc) * 128
                    eT = sb.tile([128, w], BF, tag=f"eT{kc}", bufs=1, name=f"eT{kc}")
                    eTs.append(eT)
                    off = 0
                    while off < w:
                        cw = min(512, w - off)
                        sp = pst([128, 512], "mm", 3)
                        nc.tensor.matmul(
                            sp[:, 0:cw],
                            qk_t[qrow:qrow + 64, km, kc * 128:(kc + 1) * 128],
                            qk_t[qrow:qrow + 64, qm, kc * 128 + off: kc * 128 + off + cw],
                            start=True, stop=True)
                        if off == 0:
                            nc.vector.tensor_add(sp[:, 0:128], sp[:, 0:128], mask_t[:])
                        nc.scalar.activation(eT[:, off:off + cw], sp[:, 0:cw], ActF.Exp)
                        off += cw
                for qg in range(2):
                    yp = pst([65, 512], "small", 3)
                    kcs = [kc for kc in range(TC) if kc * 128 < qg * 512 + 512]
                    for j, kc in enumerate(kcs):
                        tq_lo = max(qg * 512, kc * 128)
                        wdt = qg * 512 + 512 - tq_lo
                        nc.tensor.matmul(
                            yp[:, tq_lo - qg * 512: tq_lo - qg * 512 + wdt],
                            v_t[:, kc, hh * 65:(hh + 1) * 65],
                            eTs[kc][:, tq_lo - kc * 128: tq_lo - kc * 128 + wdt],
                            start=(j == 0), stop=(j == len(kcs) - 1),
                            skip_group_check=True)
                    linv = sbt([1, 512], F32, "linv", 2)
                    nc.vector.reciprocal(linv[:], yp[64:65, :])
                    linv_b = sbt([64, 512], F32, "linv_b", 2)
                    nc.gpsimd.partition_broadcast(linv_b[:], linv[:])
                    nc.vector.tensor_mul(
                        y_t[qrow:qrow + 64, qm, qg * 512:(qg + 1) * 512],
                        yp[0:64, :], linv_b[:])

            # out-projection -> feature-major partial, AllReduce, residual add
            att_sbs = {}
            for m in range(NCH):
                ao = sbt([128, T], BF, "arst", 2)
                for n in range(2):
                    op = pst([128, 512], "mm", 3)
                    for k2 in range(2):
                        nc.tensor.matmul(op[:], wo_t[:, k2, m * 128:(m + 1) * 128],
                                         y_t[:, k2, n * 512:(n + 1) * 512],
                                         start=(k2 == 0), stop=(k2 == 1))
                    nc.scalar.activation(ao[:, n * 512:(n + 1) * 512], op[:],
                                         ActF.Identity, bias=bo_t[:, li, m:m + 1])
                att_sbs[m] = ao
            allreduce_add(lambda m: att_sbs[m], arin_a[li], arout_a[li], bo_t, li)

            # ---------------- FFN ----------------
            h2_t = sbt([128, NCH, T], BF, "h", 1)
            layernorm(h2_t, li)
            w1_t = sbt([128, NCH, FF], BF, "w1", 1)
            w2_t = sbt([128, FCH, D], BF, "w2", 1)
            nc.sync.dma_start(out=w1_t[:], in_=w1[li].rearrange("(c p) n -> p c n", p=128))
            nc.sync.dma_start(out=w2_t[:], in_=w2[li].rearrange("(c p) n -> p c n", p=128))
            g_t = sbt([128, FCH, T], BF, "g", 1)
            for m in range(FCH):
                for n in range(2):
                    gp = pst([128, 512], "mm", 3)
                    for c in range(NCH):
                        nc.tensor.matmul(gp[:], w1_t[:, c, m * 128:(m + 1) * 128],
                                         h2_t[:, c, n * 512:(n + 1) * 512],
                                         start=(c == 0), stop=(c == NCH - 1))
                    nc.scalar.activation(g_t[:, m, n * 512:(n + 1) * 512], gp[:],
                                         ActF.Gelu, bias=b1_t[:, li, m:m + 1])
            ffn_sbs = {}
            for m in range(NCH):
                fo = sbt([128, T], BF, "arst", 2)
                for n in range(2):
                    fp = pst([128, 512], "mm", 3)
                    for c2 in range(FCH):
                        nc.tensor.matmul(fp[:], w2_t[:, c2, m * 128:(m + 1) * 128],
                                         g_t[:, c2, n * 512:(n + 1) * 512],
                                         start=(c2 == 0), stop=(c2 == FCH - 1))
                    nc.scalar.activation(fo[:, n * 512:(n + 1) * 512], fp[:],
                                         ActF.Identity, bias=b2_t[:, li, m:m + 1])
                ffn_sbs[m] = fo
            allreduce_add(lambda m: ffn_sbs[m], arin_f[li], arout_f[li], b2_t, li)

        # ---------------- final LN + LM head ----------------
        xf_t = sbt([128, NCH, T], BF, "h", 1)
        layernorm(xf_t, L)
        for vc in range(NVC):
            wl_t = sbt([128, NCH, VCW], BF, "w1", 1)
            nc.sync.dma_start(
                out=wl_t[:],
                in_=wl_d.rearrange("(c p) v -> p c v", p=128)[:, :, vc * VCW:(vc + 1) * VCW])
            blr_t = sbt([1, VCW], BF, "blr", 2)
            nc.sync.dma_start(out=blr_t[:], in_=blr_d[0:1, vc * VCW:(vc + 1) * VCW])
            for m in range(TC):
                lp = pst([128, VCW], "mm", 3)
                nc.tensor.matmul(lp[:], ones_r[0:1, :],
                                 blr_t[0:1, :],
                                 start=True, stop=False)
                for c in range(NCH):
                    nc.tensor.matmul(lp[:], xf_t[:, c, m * 128:(m + 1) * 128],
                                     wl_t[:, c, :],
                                     start=False, stop=(c == NCH - 1))
                lo = sbt([128, VCW], F32, "lo", 3)
                nc.scalar.copy(lo[:], lp[:])
                nc.sync.dma_start(
                    out=out_d[m * 128:(m + 1) * 128, vc * VCW:(vc + 1) * VCW],
                    in_=lo[:])

    nc.finalize()
    return nc


def _prep_inputs(tokens, emb, pos_emb, ln1_s, ln1_b, Wq, bq, Wk, bk, Wv, bv,
                 Wo, bo, ln2_s, ln2_b, W1, b1, W2, b2, lnf_s, lnf_b, Wl, bl):
    f = lambda a: np.asarray(a, np.float32)
    tokens = np.asarray(tokens)
    emb, pos_emb = f(emb), f(pos_emb)
    # mask (s^T orientation): mask[tk, tq] = MASKVAL where tk > tq
    mask = np.where(np.arange(128)[:, None] > np.arange(128)[None, :],
                    np.float32(MASKVAL), np.float32(0.0))
    x0s = []
    for b in range(B):
        x0 = emb[tokens[b]] + pos_emb[:T]
        x0s.append(np.ascontiguousarray(x0.T, dtype=np.float32))

    scl = np.float32(1.0 / np.sqrt(DH))
    per_tp = []
    for tp in range(TP):
        qs = slice(tp * QH, (tp + 1) * QH)
        fs = slice(tp * FF, (tp + 1) * FF)
        vs = slice(tp * VS, (tp + 1) * VS)
        m = {}
        bqk_l, bvr_l, bo_l, b1_l, b2_l = [], [], [], [], []
        for i in range(L):
            s1 = f(ln1_s)[i][:, None]
            b1v = f(ln1_b)[i]
            Wq_ = (f(Wq)[i] * s1)[:, qs] * scl
            Wk_ = (f(Wk)[i] * s1)[:, qs]
            Wv_ = (f(Wv)[i] * s1)[:, qs]
            bq_ = (f(bq)[i][qs] + b1v @ f(Wq)[i][:, qs]) * scl
            bk_ = f(bk)[i][qs] + b1v @ f(Wk)[i][:, qs]
            bv_ = f(bv)[i][qs] + b1v @ f(Wv)[i][:, qs]
            m[f"wqk{i}"] = np.ascontiguousarray(
                np.concatenate([Wq_, Wk_], 1), dtype=bf16)
            wv_aug = np.zeros((D, VW), np.float32)
            bvr = np.zeros(VW, np.float32)
            for hh in range(NH):
                wv_aug[:, hh * 65:hh * 65 + 64] = Wv_[:, hh * 64:(hh + 1) * 64]
                bvr[hh * 65:hh * 65 + 64] = bv_[hh * 64:(hh + 1) * 64]
                bvr[hh * 65 + 64] = 1.0
            m[f"wv{i}"] = wv_aug.astype(bf16)
            m[f"wo{i}"] = np.ascontiguousarray(f(Wo)[i][qs, :], dtype=bf16)
            s2 = f(ln2_s)[i][:, None]
            b2v = f(ln2_b)[i]
            W1_ = (f(W1)[i] * s2)[:, fs]
            m[f"w1{i}"] = np.ascontiguousarray(W1_, dtype=bf16)
            m[f"w2{i}"] = np.ascontiguousarray(f(W2)[i][fs, :], dtype=bf16)
            bqk_l.append(np.concatenate([bq_, bk_]))
            bvr_l.append(bvr)
            bo_l.append(f(bo)[i] / TP)
            b1_l.append(f(b1)[i][fs] + b2v @ f(W1)[i][:, fs])
            b2_l.append(f(b2)[i] / TP)
        m["bqk"] = np.stack(bqk_l).astype(np.float32)
        m["bvr"] = np.stack(bvr_l).astype(bf16)
        m["bo"] = np.stack(bo_l).astype(np.float32)
        m["b1"] = np.stack(b1_l).astype(np.float32)
        m["b2"] = np.stack(b2_l).astype(np.float32)
        Wl_ = (f(Wl) * f(lnf_s)[:, None])[:, vs]
        m["wl"] = np.ascontiguousarray(Wl_, dtype=bf16)
        m["blr"] = (f(bl)[vs] + f(lnf_b) @ f(Wl)[:, vs])[None, :].astype(bf16)
        m["maskT"] = mask
        per_tp.append(m)

    in_maps = []
    for core in range(8):
        b, tp = core // TP, core % TP
        m = dict(per_tp[tp])
        m["x0"] = x0s[b]
        in_maps.append(m)
    return in_maps


def kernel(**inputs):
    if "nc" not in _CACHE:
        _CACHE["nc"] = _build_nc()
    nc = _CACHE["nc"]
    in_maps = _prep_inputs(**inputs)
    trace = bool(int(os.environ.get("KERNEL_TRACE", "0")))
    res = run_bass_kernel_spmd(nc, in_maps, list(range(8)), trace=trace)
    _CACHE["last_exec_ns"] = res.exec_time_ns
    logits = np.empty((B, T, V), np.float32)
    for core in range(8):
        b, tp = core // TP, core % TP
        logits[b, :, tp * VS:(tp + 1) * VS] = res.results[core]["out"]
    return logits


# revision 15
# speedup vs baseline: 2108.9372x; 2108.9372x over previous
"""GPT forward on 8 Trainium2 NeuronCores.

Sharding: DP=2 (batch) x TP=4 (heads / FFN / vocab).
Cores 0-3 run batch 0, cores 4-7 batch 1; within each group of 4 the
attention heads (16 -> 4/core), FFN hidden (4096 -> 1024/core) and the
LM-head vocab (32000 -> 8000/core) are sharded Megatron-style with a
bf16 AllReduce after the attention out-projection and the FFN
down-projection.

On-device layout: the residual stream x lives feature-major ([d, t]) in
fp32 SBUF. All matmuls run in bf16 with fp32 PSUM accumulation.
LayerNorm statistics are computed with ones-vector matmuls on the
tensor engine (cross-partition sums); scale/bias of every LayerNorm are
folded into the following weights on the host. Attention computes the
score matrix transposed (s^T = k^T-major) so that no on-chip transposes
are needed anywhere; the softmax denominator comes for free from an
extra ones-column appended to V, and 1/l is applied per-head with a
partition-broadcast + one DVE multiply. exp() runs without max
subtraction (scores are O(1) by construction for this model family).
"""
import sys

sys.path.insert(0, "/opt/trn_rl_repo")

import os
import numpy as np
import ml_dtypes

import concourse.bass as bass
import concourse.mybir as mybir
import concourse.tile as tile
from concourse import bacc
from concourse.bass_utils import run_bass_kernel_spmd

BF = mybir.dt.bfloat16
F32 = mybir.dt.float32
bf16 = ml_dtypes.bfloat16

B, T, D, H, V = 2, 1024, 1024, 16, 32000
L = int(os.environ.get("KERNEL_LAYERS", "8"))
DH = D // H              # 64
TP, DP = 4, 2
NH = H // TP             # 4 heads per core
QH = NH * DH             # 256 q/k/v width per core
VW = NH * 65             # v_aug width (64 cols + ones col per head)
FF = 4 * D // TP         # 1024 ffn shard
VS = V // TP             # 8000 vocab shard
NCH = D // 128           # 8 d-chunks
FCH = FF // 128          # 8 ffn chunks
TC = T // 128            # 8 t-chunks
NVC = 16                 # lm-head vocab chunks
VCW = VS // NVC          # 500
EPS = 1e-5
MASKVAL = -200.0

ActF = mybir.ActivationFunctionType
Alu = mybir.AluOpType

_CACHE = {}


def _build_nc():
    nc = bacc.Bacc()

    x0 = nc.declare_dram_parameter("x0", [D, T], F32, isOutput=False)
    maskT = nc.declare_dram_parameter("maskT", [128, 128], F32, isOutput=False)
    wqk = [nc.declare_dram_parameter(f"wqk{i}", [D, 2 * QH], BF, isOutput=False) for i in range(L)]
    wv = [nc.declare_dram_parameter(f"wv{i}", [D, VW], BF, isOutput=False) for i in range(L)]
    wo = [nc.declare_dram_parameter(f"wo{i}", [QH, D], BF, isOutput=False) for i in range(L)]
    w1 = [nc.declare_dram_parameter(f"w1{i}", [D, FF], BF, isOutput=False) for i in range(L)]
    w2 = [nc.declare_dram_parameter(f"w2{i}", [FF, D], BF, isOutput=False) for i in range(L)]
    bqk_d = nc.declare_dram_parameter("bqk", [L, 2 * QH], F32, isOutput=False)
    bvr_d = nc.declare_dram_parameter("bvr", [L, VW], BF, isOutput=False)
    bo_d = nc.declare_dram_parameter("bo", [L, D], F32, isOutput=False)
    b1_d = nc.declare_dram_parameter("b1", [L, FF], F32, isOutput=False)
    b2_d = nc.declare_dram_parameter("b2", [L, D], F32, isOutput=False)
    wl_d = nc.declare_dram_parameter("wl", [D, VS], BF, isOutput=False)
    blr_d = nc.declare_dram_parameter("blr", [1, VS], BF, isOutput=False)
    out_d = nc.declare_dram_parameter("out", [T, VS], F32, isOutput=True)

    arin_a = [nc.dram_tensor(f"arin_a{i}", [D, T], BF) for i in range(L)]
    arout_a = [nc.dram_tensor(f"arout_a{i}", [D, T], BF) for i in range(L)]
    arin_f = [nc.dram_tensor(f"arin_f{i}", [D, T], BF) for i in range(L)]
    arout_f = [nc.dram_tensor(f"arout_f{i}", [D, T], BF) for i in range(L)]
    RG = [[0, 1, 2, 3], [4, 5, 6, 7]]

    from contextlib import ExitStack
    with tile.TileContext(nc) as tc, ExitStack() as _stk:
        sb = _stk.enter_context(tc.tile_pool(name="sb", bufs=1))
        ps = _stk.enter_context(tc.tile_pool(name="ps", bufs=1, space="PSUM"))

        def sbt(shape, dt, tag, bufs):
            return sb.tile(shape, dt, tag=tag, bufs=bufs, name=tag)

        def pst(shape, tag, bufs):
            return ps.tile(shape, F32, tag=tag, bufs=bufs, name=tag)

        # ---- persistent tiles / constants ----
        x_t = sbt([128, NCH, T], F32, "x", 1)            # residual, feature-major
        ones_r = sbt([1, 128], BF, "ones_r", 1)          # ones row (K=1 lhsT)
        ones_c = sbt([128, 1], BF, "ones_c", 1)          # ones col (stats lhsT)
        mask_t = sbt([128, 128], F32, "mask", 1)
        eps_t = sbt([1, 1], F32, "eps", 1)
        bqk_t = sbt([128, L, 4], F32, "bqk", 1)
        bvr_t = sbt([1, L, VW], BF, "bvr", 1)
        bo_t = sbt([128, L, NCH], F32, "bo", 1)
        b1_t = sbt([128, L, FCH], F32, "b1", 1)
        b2_t = sbt([128, L, NCH], F32, "b2", 1)

        nc.vector.memset(ones_r[:], 1.0)
        nc.vector.memset(ones_c[:], 1.0)
        nc.vector.memset(eps_t[:], EPS)
        nc.sync.dma_start(out=mask_t[:], in_=maskT[:])
        nc.sync.dma_start(out=x_t[:], in_=x0.rearrange("(c p) t -> p c t", p=128))
        nc.sync.dma_start(out=bqk_t[:], in_=bqk_d.rearrange("l (m p) -> p l m", p=128))
        nc.sync.dma_start(out=bvr_t[:], in_=bvr_d.rearrange("(o l) w -> o l w", o=1))
        nc.sync.dma_start(out=bo_t[:], in_=bo_d.rearrange("l (m p) -> p l m", p=128))
        nc.sync.dma_start(out=b1_t[:], in_=b1_d.rearrange("l (m p) -> p l m", p=128))
        nc.sync.dma_start(out=b2_t[:], in_=b2_d.rearrange("l (m p) -> p l m", p=128))

        def layernorm(h_t, li):
            """LN of x into h_t ([128, NCH, T] bf16), no scale/bias."""
            stat = pst([65, T], "stat", 1)  # p0 = sum(x), p32 = sum(x^2), p64 = scratch
            for c in range(NCH):
                xb = sbt([128, T], BF, "xb", 2)
                x2 = sbt([128, T], BF, "x2", 2)
                nc.gpsimd.tensor_copy(xb[:], x_t[:, c, :])
                nc.scalar.square(x2[:], x_t[:, c, :])
                for n in range(2):
                    nc.tensor.matmul(stat[0:1, n * 512:(n + 1) * 512], ones_c[:],
                                     xb[:, n * 512:(n + 1) * 512],
                                     start=(c == 0), stop=(c == NCH - 1),
                                     skip_group_check=True)
                    nc.tensor.matmul(stat[32:33, n * 512:(n + 1) * 512], ones_c[:],
                                     x2[:, n * 512:(n + 1) * 512],
                                     start=(c == 0), stop=(c == NCH - 1),
                                     skip_group_check=True)
            nm = sbt([1, T], F32, "ln_nm", 1)     # -mean
            msq = sbt([1, T], F32, "ln_msq", 1)   # E[x^2] -> var -> std
            rstd = sbt([1, T], F32, "ln_rstd", 1)
            sqm = stat[64:65, :]                  # psum scratch: mean^2, -mean*rstd
            nc.vector.tensor_scalar_mul(nm[:], stat[0:1, :], -1.0 / D)
            nc.vector.tensor_scalar_mul(msq[:], stat[32:33, :], 1.0 / D)
            nc.vector.tensor_mul(sqm, nm[:], nm[:])
            nc.vector.tensor_sub(msq[:], msq[:], sqm)
            nc.scalar.activation(msq[:], msq[:], ActF.Sqrt, bias=eps_t[:])
            nc.vector.reciprocal(rstd[:], msq[:])
            nc.vector.tensor_mul(sqm, nm[:], rstd[:])
            stb = sbt([1, 2, T], BF, "lnstb", 1)
            nc.vector.tensor_copy(stb[0:1, 0, :], sqm)
            nc.vector.tensor_copy(stb[0:1, 1, :], rstd[:])
            nm_b = sbt([128, T], BF, "nm_b", 2)
            rs_b = sbt([128, T], BF, "rs_b", 2)
            nc.gpsimd.partition_broadcast(nm_b[:], stb[0:1, 0, :])
            nc.gpsimd.partition_broadcast(rs_b[:], stb[0:1, 1, :])
            for c in range(NCH):
                tmp = sbt([128, T], BF, "lntmp", 2)
                nc.vector.tensor_mul(tmp[:], x_t[:, c, :], rs_b[:])
                nc.vector.tensor_add(h_t[:, c, :], tmp[:], nm_b[:])

        def allreduce_add(src_sb_fn, arin, arout, bias_t, li):
            """Evacuate 8 fm psum chunks (via src_sb_fn -> bf16 tiles), DMA to
            arin, AllReduce into arout, then x += arout."""
            for m in range(NCH):
                st = src_sb_fn(m)
                nc.sync.dma_start(out=arin[m * 128:(m + 1) * 128, :], in_=st[:])
            nc.gpsimd.collective_compute(
                "AllReduce", Alu.add, replica_groups=RG,
                ins=[arin[:]], outs=[arout[:]])
            for c in range(NCH):
                aro = sbt([128, T], BF, "arld", 2)
                nc.sync.dma_start(out=aro[:], in_=arout.rearrange("(c p) t -> p c t", p=128)[:, c, :])
                nc.vector.tensor_add(x_t[:, c, :], x_t[:, c, :], aro[:])

        for li in range(L):
            # ---------------- attention ----------------
            h_t = sbt([128, NCH, T], BF, "h", 1)
            layernorm(h_t, li)

            wqk_t = sbt([128, NCH, 2 * QH], BF, "wqk", 1)
            wv_t = sbt([128, NCH, VW], BF, "wv", 1)
            wo_t = sbt([128, 2, D], BF, "wo", 1)
            nc.sync.dma_start(out=wqk_t[:], in_=wqk[li].rearrange("(c p) n -> p c n", p=128))
            nc.sync.dma_start(out=wv_t[:], in_=wv[li].rearrange("(c p) n -> p c n", p=128))
            nc.sync.dma_start(out=wo_t[:], in_=wo[li].rearrange("(c p) n -> p c n", p=128))

            # q/k feature-major: qk_t rows x [m, t]; m=0,1 -> q (heads 2m..),
            # m=2,3 -> k
            qk_t = sbt([128, 4, T], BF, "qk", 1)
            for m in range(4):
                for n in range(2):
                    qp = pst([128, 512], "mm", 3)
                    for c in range(NCH):
                        nc.tensor.matmul(qp[:], wqk_t[:, c, m * 128:(m + 1) * 128],
                                         h_t[:, c, n * 512:(n + 1) * 512],
                                         start=(c == 0), stop=(c == NCH - 1))
                    nc.scalar.activation(qk_t[:, m, n * 512:(n + 1) * 512], qp[:],
                                         ActF.Identity, bias=bqk_t[:, li, m:m + 1])
            # v token-major (with ones col per head): [t-chunk][128, VW]
            v_t = sbt([128, TC, VW], BF, "v", 1)
            for tc_ in range(TC):
                vp = pst([128, VW], "small", 3)
                nc.tensor.matmul(vp[:], ones_r[0:1, :], bvr_t[0:1, li, :],
                                 start=True, stop=False)
                for c in range(NCH):
                    nc.tensor.matmul(vp[:], h_t[:, c, tc_ * 128:(tc_ + 1) * 128],
                                     wv_t[:, c, :],
                                     start=False, stop=(c == NCH - 1))
                nc.vector.tensor_copy(v_t[:, tc_, :], vp[:])

            # scores^T + softmax + attnV, per head
            y_t = sbt([128, 2, T], BF, "y", 1)
            for hh in range(NH):
                qrow, qm = (hh % 2) * 64, hh // 2
                km = 2 + hh // 2
                eTs = []
                for kc in range(TC):
                    w = (TC - kc) * 128
                    eT = sb.tile([128, w], BF, tag=f"eT{kc}", bufs=1, name=f"eT{kc}")
                    eTs.append(eT)
                    off = 0
                    while off < w:
                        cw = min(512, w - off)
                        sp = pst([128, 512], "mm", 3)
                        nc.tensor.matmul(
                            sp[:, 0:cw],
                            qk_t[qrow:qrow + 64, km, kc * 128:(kc + 1) * 128],
                            qk_t[qrow:qrow + 64, qm, kc * 128 + off: kc * 128 + off + cw],
                            start=True, stop=True)
                        if off == 0:
                            nc.vector.tensor_add(sp[:, 0:128], sp[:, 0:128], mask_t[:])
                        nc.scalar.activation(eT[:, off:off + cw], sp[:, 0:cw], ActF.Exp)
                        off += cw
                for qg in range(2):
                    yp = pst([65, 512], "small", 3)
                    kcs = [kc for kc in range(TC) if kc * 128 < qg * 512 + 512]
                    for j, kc in enumerate(kcs):
                        tq_lo = max(qg * 512, kc * 128)
                        wdt = qg * 512 + 512 - tq_lo
                        nc.tensor.matmul(
                            yp[:, tq_lo - qg * 512: tq_lo - qg * 512 + wdt],
                            v_t[:, kc, hh * 65:(hh + 1) * 65],
                            eTs[kc][:, tq_lo - kc * 128: tq_lo - kc * 128 + wdt],
                            start=(j == 0), stop=(j == len(kcs) - 1),
                            skip_group_check=True)
                    linv = sbt([1, 512], F32, "linv", 2)
                    nc.vector.reciprocal(linv[:], yp[64:65, :])
                    linv_b = sbt([64, 512], F32, "linv_b", 2)
                    nc.gpsimd.partition_broadcast(linv_b[:], linv[:])
                    nc.vector.tensor_mul(
                        y_t[qrow:qrow + 64, qm, qg * 512:(qg + 1) * 512],
                        yp[0:64, :], linv_b[:])

            # out-projection -> feature-major partial, AllReduce, residual add
            att_sbs = {}
            for m in range(NCH):
                ao = sbt([128, T], BF, "arst", 2)
                for n in range(2):
                    op = pst([128, 512], "mm", 3)
                    for k2 in range(2):
                        nc.tensor.matmul(op[:], wo_t[:, k2, m * 128:(m + 1) * 128],
                                         y_t[:, k2, n * 512:(n + 1) * 512],
                                         start=(k2 == 0), stop=(k2 == 1))
                    nc.scalar.activation(ao[:, n * 512:(n + 1) * 512], op[:],
                                         ActF.Identity, bias=bo_t[:, li, m:m + 1])
                att_sbs[m] = ao
            allreduce_add(lambda m: att_sbs[m], arin_a[li], arout_a[li], bo_t, li)

            # ---------------- FFN ----------------
            h2_t = sbt([128, NCH, T], BF, "h", 1)
            layernorm(h2_t, li)
            w1_t = sbt([128, NCH, FF], BF, "w1", 1)
            w2_t = sbt([128, FCH, D], BF, "w2", 1)
            nc.sync.dma_start(out=w1_t[:], in_=w1[li].rearrange("(c p) n -> p c n", p=128))
            nc.sync.dma_start(out=w2_t[:], in_=w2[li].rearrange("(c p) n -> p c n", p=128))
            g_t = sbt([128, FCH, T], BF, "g", 1)
            for m in range(FCH):
                for n in range(2):
                    gp = pst([128, 512], "mm", 3)
                    for c in range(NCH):
                        nc.tensor.matmul(gp[:], w1_t[:, c, m * 128:(m + 1) * 128],
                                         h2_t[:, c, n * 512:(n + 1) * 512],
                                         start=(c == 0), stop=(c == NCH - 1))
                    nc.scalar.activation(g_t[:, m, n * 512:(n + 1) * 512], gp[:],
                                         ActF.Gelu, bias=b1_t[:, li, m:m + 1])
            ffn_sbs = {}
            for m in range(NCH):
                fo = sbt([128, T], BF, "arst", 2)
                for n in range(2):
                    fp = pst([128, 512], "mm", 3)
                    for c2 in range(FCH):
                        nc.tensor.matmul(fp[:], w2_t[:, c2, m * 128:(m + 1) * 128],
                                         g_t[:, c2, n * 512:(n + 1) * 512],
                                         start=(c2 == 0), stop=(c2 == FCH - 1))
                    nc.scalar.activation(fo[:, n * 512:(n + 1) * 512], fp[:],
                                         ActF.Identity, bias=b2_t[:, li, m:m + 1])
                ffn_sbs[m] = fo
            allreduce_add(lambda m: ffn_sbs[m], arin_f[li], arout_f[li], b2_t, li)

        # ---------------- final LN + LM head ----------------
        xf_t = sbt([128, NCH, T], BF, "h", 1)
        layernorm(xf_t, L)
        for vc in range(NVC):
            wl_t = sbt([128, NCH, VCW], BF, "w1", 1)
            nc.sync.dma_start(
                out=wl_t[:],
                in_=wl_d.rearrange("(c p) v -> p c v", p=128)[:, :, vc * VCW:(vc + 1) * VCW])
            blr_t = sbt([1, VCW], BF, "blr", 2)
            nc.sync.dma_start(out=blr_t[:], in_=blr_d[0:1, vc * VCW:(vc + 1) * VCW])
            for m in range(TC):
                lp = pst([128, VCW], "mm", 3)
                nc.tensor.matmul(lp[:], ones_r[0:1, :],
                                 blr_t[0:1, :],
                                 start=True, stop=False)
                for c in range(NCH):
                    nc.tensor.matmul(lp[:], xf_t[:, c, m * 128:(m + 1) * 128],
                                     wl_t[:, c, :],
                                     start=False, stop=(c == NCH - 1))
                lo = sbt([128, VCW], F32, "lo", 3)
                nc.scalar.copy(lo[:], lp[:])
                nc.sync.dma_start(
                    out=out_d[m * 128:(m + 1) * 128, vc * VCW:(vc + 1) * VCW],
                    in_=lo[:])

    nc.finalize()
    return nc


def _prep_inputs(tokens, emb, pos_emb, ln1_s, ln1_b, Wq, bq, Wk, bk, Wv, bv,
                 Wo, bo, ln2_s, ln2_b, W1, b1, W2, b2, lnf_s, lnf_b, Wl, bl):
    f = lambda a: np.asarray(a, np.float32)
    tokens = np.asarray(tokens)
    emb, pos_emb = f(emb), f(pos_emb)
    # mask (s^T orientation): mask[tk, tq] = MASKVAL where tk > tq
    mask = np.where(np.arange(128)[:, None] > np.arange(128)[None, :],
                    np.float32(MASKVAL), np.float32(0.0))
    x0s = []
    for b in range(B):
        x0 = emb[tokens[b]] + pos_emb[:T]
        x0s.append(np.ascontiguousarray(x0.T, dtype=np.float32))

    scl = np.float32(1.0 / np.sqrt(DH))
    per_tp = []
    for tp in range(TP):
        qs = slice(tp * QH, (tp + 1) * QH)
        fs = slice(tp * FF, (tp + 1) * FF)
        vs = slice(tp * VS, (tp + 1) * VS)
        m = {}
        bqk_l, bvr_l, bo_l, b1_l, b2_l = [], [], [], [], []
        for i in range(L):
            s1 = f(ln1_s)[i][:, None]
            b1v = f(ln1_b)[i]
            Wq_ = (f(Wq)[i] * s1)[:, qs] * scl
            Wk_ = (f(Wk)[i] * s1)[:, qs]
            Wv_ = (f(Wv)[i] * s1)[:, qs]
            bq_ = (f(bq)[i][qs] + b1v @ f(Wq)[i][:, qs]) * scl
            bk_ = f(bk)[i][qs] + b1v @ f(Wk)[i][:, qs]
            bv_ = f(bv)[i][qs] + b1v @ f(Wv)[i][:, qs]
            m[f"wqk{i}"] = np.ascontiguousarray(
                np.concatenate([Wq_, Wk_], 1), dtype=bf16)
            wv_aug = np.zeros((D, VW), np.float32)
            bvr = np.zeros(VW, np.float32)
            for hh in range(NH):
                wv_aug[:, hh * 65:hh * 65 + 64] = Wv_[:, hh * 64:(hh + 1) * 64]
                bvr[hh * 65:hh * 65 + 64] = bv_[hh * 64:(hh + 1) * 64]
                bvr[hh * 65 + 64] = 1.0
            m[f"wv{i}"] = wv_aug.astype(bf16)
            m[f"wo{i}"] = np.ascontiguousarray(f(Wo)[i][qs, :], dtype=bf16)
            s2 = f(ln2_s)[i][:, None]
            b2v = f(ln2_b)[i]
            W1_ = (f(W1)[i] * s2)[:, fs]
            m[f"w1{i}"] = np.ascontiguousarray(W1_, dtype=bf16)
            m[f"w2{i}"] = np.ascontiguousarray(f(W2)[i][fs, :], dtype=bf16)
            bqk_l.append(np.concatenate([bq_, bk_]))
            bvr_l.append(bvr)
            bo_l.append(f(bo)[i] / TP)
            b1_l.append(f(b1)[i][fs] + b2v @ f(W1)[i][:, fs])
            b2_l.append(f(b2)[i] / TP)
        m["bqk"] = np.stack(bqk_l).astype(np.float32)
        m["bvr"] = np.stack(bvr_l).astype(bf16)
        m["bo"] = np.stack(bo_l).astype(np.float32)
        m["b1"] = np.stack(b1_l).astype(np.float32)
        m["b2"] = np.stack(b2_l).astype(np.float32)
        Wl_ = (f(Wl) * f(lnf_s)[:, None])[:, vs]
        m["wl"] = np.ascontiguousarray(Wl_, dtype=bf16)
        m["blr"] = (f(bl)[vs] + f(lnf_b) @ f(Wl)[:, vs])[None, :].astype(bf16)
        m["maskT"] = mask
        per_tp.append(m)

    in_maps = []
    for core in range(8):
        b, tp = core // TP, core % TP
        m = dict(per_tp[tp])
        m["x0"] = x0s[b]
        in_maps.append(m)
    return in_maps


def kernel(**inputs):
    if "nc" not in _CACHE:
        _CACHE["nc"] = _build_nc()
    nc = _CACHE["nc"]
    in_maps = _prep_inputs(**inputs)
    trace = bool(int(os.environ.get("KERNEL_TRACE", "0")))
    res = run_bass_kernel_spmd(nc, in_maps, list(range(8)), trace=trace)
    _CACHE["last_exec_ns"] = res.exec_time_ns
    logits = np.empty((B, T, V), np.float32)
    for core in range(8):
        b, tp = core // TP, core % TP
        logits[b, :, tp * VS:(tp + 1) * VS] = res.results[core]["out"]
    return logits


def _timed_run(nc, in_maps, iters=4):
    """Mirror bass2jax.run_bass_via_pjrt but pre-stage inputs on device and
    time repeated executions (returns list of per-iter seconds)."""
    import time
    import jax
    import jax.numpy as jnp
    from jax.sharding import Mesh, PartitionSpec
    from jax.experimental.shard_map import shard_map
    from concourse import bass2jax
    from concourse.bass2jax import _bass_exec_p, install_neuronx_cc_hook, partition_id_tensor
    import concourse.mybir as mybir

    install_neuronx_cc_hook()
    n_cores = len(in_maps)
    partition_name = nc.partition_id_tensor.name if nc.partition_id_tensor else None
    in_names, out_names, out_avals, zero_outs = [], [], [], []
    for alloc in nc.m.functions[0].allocations:
        if not isinstance(alloc, mybir.MemoryLocationSet):
            continue
        name = alloc.memorylocations[0].name
        if alloc.kind == "ExternalInput":
            if name != partition_name:
                in_names.append(name)
        elif alloc.kind == "ExternalOutput":
            out_names.append(name)
            shape = tuple(alloc.tensor_shape)
            dtype = mybir.dt.np(alloc.dtype)
            out_avals.append(jax.core.ShapedArray(shape, dtype))
            zero_outs.append(np.zeros(shape, dtype))
    n_params = len(in_names)
    n_outs = len(out_avals)
    all_names = in_names + out_names + ([partition_name] if partition_name else [])
    donate = tuple(range(n_params, n_params + n_outs))

    def _body(*args):
        operands = list(args)
        if partition_name is not None:
            operands.append(partition_id_tensor())
        outs = _bass_exec_p.bind(
            *operands, out_avals=tuple(out_avals),
            in_names=tuple(all_names[:n_params]
                           + out_names + ([partition_name] if partition_name else [])),
            out_names=tuple(out_names),
            lowering_input_output_aliases=(),
            sim_require_finite=True, sim_require_nnan=True, nc=nc)
        return tuple(outs)

    devices = jax.devices()[:n_cores]
    mesh = Mesh(np.array(devices), ("core",))
    in_specs = (PartitionSpec("core"),) * (n_params + n_outs)
    out_specs = (PartitionSpec("core"),) * len(out_names)
    sharded = jax.jit(
        shard_map(_body, mesh=mesh, in_specs=in_specs, out_specs=out_specs,
                  check_rep=False),
        donate_argnums=donate, keep_unused=True)
    sh = jax.sharding.NamedSharding(mesh, PartitionSpec("core"))
    concat_in = [
        jax.device_put(
            np.concatenate([np.asarray(in_maps[c][in_names[i]]) for c in range(n_cores)], axis=0),
            sh)
        for i in range(n_params)
    ]
    times = []
    outs = None
    for it in range(iters):
        concat_zeros = [
            jax.device_put(np.zeros((n_cores * z.shape[0], *z.shape[1:]), z.dtype), sh)
            for z in zero_outs
        ]
        for z in concat_zeros:
            z.block_until_ready()
        t0 = time.perf_counter()
        outs = sharded(*concat_in, *concat_zeros)
        for o in outs:
            o.block_until_ready()
        times.append(time.perf_counter() - t0)
    results = [
        {name: np.asarray(outs[i]).reshape(n_cores, *out_avals[i].shape)[c]
         for i, name in enumerate(out_names)}
        for c in range(n_cores)
    ]
    return times, results
